# revision 105
# baseline (speedup 1.0000x reference)
"""Trainium2 Bass kernel for nn_Attention (B=4, S=2048, D=2048, H=16, KV=4, HD=128).

Sharding (8 cores): data-parallel over batch (4) x tensor-parallel over
KV-head-group halves (2). Core c handles batch b=c//2 and q-heads
[8*(c%2), 8*(c%2)+8) == kv groups {2*(c%2), 2*(c%2)+1}. Each core produces a
partial output (its heads' contribution through wo); the host sums the two
partials per batch.

v2 design (449.5us/core vs the 584.2us baseline, 1.30x):
- All matmul operands in bf16 (1.0 cycles/row, same PE speed as f32r on
  TRN2, but half the DMA traffic and SBUF footprint; PSUM stays fp32).
  Adds ~0.4% relative error; total 4.7e-3 vs the 2e-2 gate.
- Stage 1 is one fused streamed loop: kv(sb) runs one s-block ahead of
  q(sb-1) so the wq DMA stream hides behind the kv matmuls; RoPE on DVE in
  [s,e] layout, PE transposes (bf16), qT/kT stores via ACT copies.
- Stage 2 scores are computed in 2-bank psum pairs so one ACT exp covers
  two k-blocks (halving ACT's per-tile overhead); probs are written
  [128, q, t]-packed bf16. The causal diagonal is handled by multiplying
  the exp'd block with a 0/1 triangle (Pool for the big qsbs, DVE for the
  small ones) instead of an additive -inf mask.
- Softmax denominators never touch PE for qsb>0: the packed probs are
  t-summed into one tile by pipelined DVE tensor_reduce chunks plus
  regioned diagonal adds, and gpsimd.partition_all_reduce folds the 128
  partitions AND broadcasts in one Pool op; a DVE reciprocal finishes the
  job. qsb0 (tiny) uses PE ones-matmul + broadcast instead.
- Each head's denominator/normalize work is DEFERRED one head so PE never
  waits on the DVE reduce chain; wo (resident in SBUF) is software-
  pipelined one qsb behind attention and interleaved per-head with a
  back-loaded schedule (h0 skipped - its att is not complete yet; late
  heads carry more blocks to cover the last heads' denominator chains).
  Output stored bf16, host sums the two TP partials per batch in fp32.
PE ~406us busy (~90%); DVE ~290us; ACT ~268us; Pool ~50us.
"""
import numpy as np

B, S, D = 4, 2048, 2048
H, KV, HD = 16, 4, 128
NREP = H // KV
SCALE = float(HD) ** -0.5

SB = S // 128          # 16 s-blocks of 128
KT = D // 128          # 16 contraction chunks for projections
QSB = S // 512         # 4 q-superblocks
SBL = S // 512         # 4 s-superblocks (stage 1 streaming)
HPC = 8                # q heads per core
GPC = 2                # kv groups per core

_compiled = {}


def _build(causal: bool):
    import concourse.bass as bass  # noqa: F401
    import concourse.tile as tile
    from concourse import bacc, mybir

    f32 = mybir.dt.float32
    f32r = mybir.dt.float32r
    bf16 = mybir.dt.bfloat16
    AF = mybir.ActivationFunctionType
    ALU = mybir.AluOpType
    AX = mybir.AxisListType

    nc = bacc.Bacc("TRN2")

    # xT: [D, S] (d-major).  wqT: [D, HPC*HD], wkvT: [D, 2*GPC*HD] (K|V),
    # woT: [HPC*HD, D] natural.  cosS/sinS: [128, SB, 64] (s-major tiles).
    xT = nc.dram_tensor("xT", [D, S], bf16, kind="ExternalInput")
    wqT = nc.dram_tensor("wqT", [D, HPC * HD], bf16, kind="ExternalInput")
    wkvT = nc.dram_tensor("wkvT", [D, 2 * GPC * HD], bf16, kind="ExternalInput")
    woT = nc.dram_tensor("woT", [HPC * HD, D], bf16, kind="ExternalInput")
    cosS = nc.dram_tensor("cosS", [128, SB, 64], f32, kind="ExternalInput")
    sinS = nc.dram_tensor("sinS", [128, SB, 64], f32, kind="ExternalInput")
    mtile = nc.dram_tensor("mtile", [128, 128], f32, kind="ExternalInput")
    tri01d = nc.dram_tensor("tri01", [128, 128], bf16, kind="ExternalInput")
    onest = nc.dram_tensor("onest", [128, 128], f32r, kind="ExternalInput")
    outT = nc.dram_tensor("outT", [D, S], bf16, kind="ExternalOutput")

    xT3 = xT.rearrange("(kt p) s -> p kt s", p=128)
    wqT3 = wqT.rearrange("(kt p) e -> p kt e", p=128)
    wkvT3 = wkvT.rearrange("(kt p) e -> p kt e", p=128)
    woT3 = woT.rearrange("(h p) d -> p h d", p=128)

    with tile.TileContext(nc) as tc:
        with tc.tile_pool(name="persist", bufs=1) as persist:
            # persistent activations (bf16)
            qT = [persist.tile([128, S], bf16, tag=f"qT{h}", name=f"qT{h}")
                  for h in range(HPC)]
            kT = [persist.tile([128, S], bf16, tag=f"kTg{g}", name=f"kTg{g}")
                  for g in range(GPC)]
            vsb = [persist.tile([128, SB, 128], bf16, tag=f"v{g}", name=f"v{g}")
                   for g in range(GPC)]
            tri01 = persist.tile([128, 128], bf16, tag="tri01")
            nc.gpsimd.dma_start(out=tri01, in_=tri01d[:, :])
            ones = persist.tile([128, 128], f32r, tag="ones")
            nc.gpsimd.dma_start(out=ones, in_=onest[:, :])
            ones_bf = persist.tile([128, 1], bf16, tag="onesbf")
            with nc.allow_low_precision(reason="ones"):
                nc.vector.tensor_copy(out=ones_bf, in_=ones[:, 0:1])

            # ------- Stage 1: projections + RoPE + PE transposes ------------
            # ([s,e] orientation like the baseline: DVE ops stay partition-
            # aligned, which the BIR verifier requires)
            s1ctx = tc.tile_pool(name="s1const", bufs=1)
            s1c = s1ctx.__enter__()
            from concourse.masks import make_identity
            ident_f = s1c.tile([128, 128], f32, tag="identf")
            make_identity(nc, ident_f)
            ident = s1c.tile([128, 128], bf16, tag="ident")
            nc.vector.tensor_copy(out=ident, in_=ident_f)
            cos_t = s1c.tile([128, SB, 64], f32, tag="cos")
            sin_t = s1c.tile([128, SB, 64], f32, tag="sin")
            nc.gpsimd.dma_start(out=cos_t, in_=cosS[:, :, :])
            nc.gpsimd.dma_start(out=sin_t, in_=sinS[:, :, :])

            wkv = s1c.tile([128, KT, 2 * GPC * HD], bf16, tag="wkv")
            wq = s1c.tile([128, KT, HPC * HD], bf16, tag="wq")
            for kt in range(0, 4):
                nc.scalar.dma_start(out=wkv[:, kt:kt + 1, :],
                                    in_=wkvT3[:, kt:kt + 1, :])
            for kt4 in range(4, KT, 4):
                nc.scalar.dma_start(out=wkv[:, kt4:kt4 + 4, :],
                                    in_=wkvT3[:, kt4:kt4 + 4, :])

            with tc.tile_pool(name="xs1", bufs=3) as xpool, \
                 tc.tile_pool(name="rs1", bufs=2) as rpool, \
                 tc.tile_pool(name="pkv1", bufs=2, space="PSUM") as pkvp, \
                 tc.tile_pool(name="pq1", bufs=2, space="PSUM") as pqp, \
                 tc.tile_pool(name="pt1", bufs=2, space="PSUM") as ptp:

                def load_xs(sb):
                    xs = xpool.tile([128, KT, 128], bf16, tag="xs")
                    for kt8 in range(0, KT, 8):
                        nc.sync.dma_start(
                            out=xs[:, kt8:kt8 + 8, :],
                            in_=xT3[:, kt8:kt8 + 8, sb * 128:(sb + 1) * 128])
                    return xs

                def rope_block(ps3, nr, sb, rtag):
                    rp = rpool.tile([128, HPC, 128], bf16, tag=rtag)
                    ev = ps3[:, 0:nr, 0:128:2]
                    od = ps3[:, 0:nr, 1:128:2]
                    cb = cos_t[:, None, sb, :].broadcast_to([128, nr, 64])
                    sn = sin_t[:, None, sb, :].broadcast_to([128, nr, 64])
                    t1 = rpool.tile([128, HPC, 64], f32, tag="t1" + rtag)
                    t2 = rpool.tile([128, HPC, 64], f32, tag="t2" + rtag)
                    with nc.allow_low_precision(reason="bf16 rope"):
                        nc.vector.tensor_tensor(
                            out=t1[:, 0:nr, :], in0=ev, in1=cb, op=ALU.mult)
                        nc.vector.tensor_tensor(
                            out=t2[:, 0:nr, :], in0=od, in1=sn, op=ALU.mult)
                        nc.vector.tensor_tensor(
                            out=rp[:, 0:nr, 0:64], in0=t1[:, 0:nr, :],
                            in1=t2[:, 0:nr, :], op=ALU.subtract)
                        nc.vector.tensor_tensor(
                            out=t1[:, 0:nr, :], in0=ev, in1=sn, op=ALU.mult)
                        nc.vector.tensor_tensor(
                            out=t2[:, 0:nr, :], in0=od, in1=cb, op=ALU.mult)
                        nc.vector.tensor_tensor(
                            out=rp[:, 0:nr, 64:128], in0=t1[:, 0:nr, :],
                            in1=t2[:, 0:nr, :], op=ALU.add)
                    return rp

                def store_T(rp, nr, sb, dsts):
                    for hh in range(nr):
                        pt = ptp.tile([128, 128], bf16, tag="pt")
                        nc.tensor.transpose(pt, rp[:, hh, :], ident)
                        with nc.allow_low_precision(reason="bf16 qkT"):
                            nc.scalar.copy(
                                out=dsts[hh][:, sb * 128:(sb + 1) * 128],
                                in_=pt)

                # staggered: kv(sb) one step ahead of q(sb-1), so the wq
                # stream (behind xs on the sync queue) has time to land
                xs_tiles = {0: load_xs(0), 1: load_xs(1)}
                for kt4 in range(0, KT, 4):
                    nc.sync.dma_start(out=wq[:, kt4:kt4 + 4, :],
                                      in_=wqT3[:, kt4:kt4 + 4, :])
                for sb in range(SB + 1):
                    if sb < SB:
                        if sb + 1 < SB and sb + 1 not in xs_tiles:
                            xs_tiles[sb + 1] = load_xs(sb + 1)
                        xs = xs_tiles[sb]
                        ps = pkvp.tile([128, 2 * GPC * HD], f32, tag="pskv")
                        for kt in range(KT):
                            nc.tensor.matmul(
                                ps[:, :], xs[:, kt, :], wkv[:, kt, :],
                                start=(kt == 0), stop=(kt == KT - 1))
                        ps3 = ps.rearrange("p (h d) -> p h d", d=128)
                        with nc.allow_low_precision(reason="bf16 v"):
                            for g in range(GPC):
                                nc.scalar.copy(out=vsb[g][:, sb, :],
                                               in_=ps3[:, GPC + g, :])
                        rp = rope_block(ps3, GPC, sb, "kv")
                        store_T(rp, GPC, sb, kT)
                    if sb >= 1:
                        qb = sb - 1
                        xs = xs_tiles[qb]
                        ps = pqp.tile([128, HPC * HD], f32, tag="psq")
                        for kt in range(KT):
                            for n0 in range(0, HPC * HD, 512):
                                nc.tensor.matmul(
                                    ps[:, n0:n0 + 512], xs[:, kt, :],
                                    wq[:, kt, n0:n0 + 512],
                                    start=(kt == 0), stop=(kt == KT - 1))
                        ps3 = ps.rearrange("p (h d) -> p h d", d=128)
                        rp = rope_block(ps3, HPC, qb, "q")
                        store_T(rp, HPC, qb, qT)
                        del xs_tiles[qb]
            s1ctx.__exit__(None, None, None)

            # ------------ Stage 2+3: attention (scoresT) + out-projection ---
            with tc.tile_pool(name="wo2", bufs=1) as wopool, \
                 tc.tile_pool(name="pr2", bufs=4) as prpool, \
                 tc.tile_pool(name="att2", bufs=2) as attpool, \
                 tc.tile_pool(name="ts2", bufs=3) as tspool, \
                 tc.tile_pool(name="tc2", bufs=2) as tcpool, \
                 tc.tile_pool(name="rr2", bufs=3) as rrpool, \
                 tc.tile_pool(name="o2", bufs=2) as opool, \
                 tc.tile_pool(name="psc", bufs=2, space="PSUM") as pscp, \
                 tc.tile_pool(name="pav", bufs=2, space="PSUM") as pavp, \
                 tc.tile_pool(name="pou", bufs=2, space="PSUM") as poup:
                wo = wopool.tile([128, HPC, D], bf16, tag="wo")
                for m4 in range(0, KT, 4):
                    nc.sync.dma_start(
                        out=wo[:, :, m4 * 128:(m4 + 4) * 128],
                        in_=woT3[:, :, m4 * 128:(m4 + 4) * 128])

                def wo_block(m, qsb, att):
                    po = poup.tile([128, 512], f32, tag="po")
                    for e in range(HPC):
                        nc.tensor.matmul(
                            po, wo[:, e, m * 128:(m + 1) * 128],
                            att[:, e, :],
                            start=(e == 0), stop=(e == HPC - 1))
                    ot = opool.tile([128, 512], bf16, tag="ot")
                    with nc.allow_low_precision(reason="bf16 out"):
                        nc.scalar.copy(out=ot, in_=po)
                    nc.sync.dma_start(
                        out=outT[m * 128:(m + 1) * 128,
                                 qsb * 512:(qsb + 1) * 512],
                        in_=ot)

                def finish_head(att, h, av, denom, kind):
                    """Denominator combine + reciprocal + normalization for
                    a head whose scores/AV/t-sums were emitted earlier.
                    kind 'tile': partition_all_reduce (Pool) of a [128,512]
                    partial-sum tile. kind 'row': a [1,512] PSUM row from PE
                    ones-matmuls, broadcast back via a PE matmul."""
                    from concourse import bass_isa
                    if kind == "tile":
                        bc = rrpool.tile([128, 512], f32r, tag="bc")
                        nc.gpsimd.partition_all_reduce(
                            bc, denom, channels=128,
                            reduce_op=bass_isa.ReduceOp.add)
                        rr = rrpool.tile([128, 512], f32r, tag="rr")
                        with nc.allow_low_precision(reason="recip"):
                            nc.vector.reciprocal(out=rr, in_=bc)
                    else:
                        rr1 = rrpool.tile([1, 512], f32r, tag="rr1")
                        with nc.allow_low_precision(reason="recip"):
                            nc.vector.reciprocal(out=rr1, in_=denom[0:1, :])
                        rrp = pavp.tile([128, 512], f32, tag="av")
                        nc.tensor.matmul(rrp, ones[0:1, :], rr1,
                                         start=True, stop=True)
                        # norm can't read two PSUM operands; stage via ACT
                        rr = rrpool.tile([128, 512], f32r, tag="rr")
                        with nc.allow_low_precision(reason="rr copy"):
                            nc.scalar.copy(out=rr, in_=rrp)
                    with nc.allow_low_precision(reason="bf16 att"):
                        nc.vector.tensor_tensor(
                            out=att[:, h, :], in0=av, in1=rr, op=ALU.mult)

                prev_att = None
                pending = None
                for qsb in range(QSB):
                    att = attpool.tile([128, HPC, 512], bf16, tag="att")
                    maxkt = (qsb + 1) * 4 if causal else SB
                    q0g = qsb * 512
                    for g in range(GPC):
                        for r in range(NREP):
                            h = g * NREP + r
                            probs = prpool.tile([128, 512, SB], bf16,
                                                tag="probs")
                            ndiag = min(4, maxkt) if causal else 0
                            nsub = maxkt - ndiag
                            tsum = tsum2 = dsr0 = None
                            if nsub > 0:
                                tsum = tspool.tile([128, 512], f32r,
                                                   tag="tsum", name="tsum")
                            if causal and nsub == 0:
                                dsr0 = poup.tile([128, 512], f32, tag="po",
                                                 name="dsr")
                            lp = nc.allow_low_precision(reason="denoms")
                            lp.__enter__()
                            # scores + exp in 2-bank pairs: one ACT exp per
                            # two t-blocks. Diagonal pairs exp full-width
                            # then get the causal triangle zeroed by a Pool
                            # multiply with tri01; the [0:ql) garbage
                            # regions are never read.
                            for t in range(0, maxkt, 2):
                                sc = pscp.tile([128, 2, 512], f32, tag="sc")
                                for j in range(2):
                                    tt = t + j
                                    ql = (max(0, tt * 128 - q0g)
                                          if causal else 0)
                                    nc.tensor.matmul(
                                        sc[:, j, ql:512],
                                        kT[g][:, tt * 128:(tt + 1) * 128],
                                        qT[h][:, q0g + ql:q0g + 512],
                                        start=True, stop=True)
                                nc.scalar.activation(
                                    out=probs[:, :, t:t + 2],
                                    in_=sc.rearrange("p t q -> p q t"),
                                    func=AF.Exp, scale=SCALE)
                                tdone = t + 2
                                # tri-mask + diag-sum engine: DVE when it
                                # is idle (small qsb), Pool when DVE is
                                # loaded with the big t-sum reduces
                                deng = nc.vector
                                teng = nc.gpsimd if nsub >= 8 else nc.vector
                                for tt in range(t, tdone):
                                    if causal and tt * 128 >= q0g:
                                        ql = tt * 128 - q0g
                                        # zero the masked (upper) triangle
                                        teng.tensor_tensor(
                                            out=probs[:, ql:ql + 128, tt],
                                            in0=probs[:, ql:ql + 128, tt],
                                            in1=tri01, op=ALU.mult)
                                if pending is not None and nsub == 0:
                                    # qsb0: flush before this head's dsr
                                    # matmuls so the pds pool can rotate
                                    finish_head(*pending)
                                    pending = None
                                # pipelined t-sum on DVE: chunk reduces
                                # sized to balance op overhead vs pipeline
                                # tail (qsb3: 6+6, qsb2: 8, qsb1: 4)
                                chunks = {4: [(0, 4)], 8: [(0, 8)],
                                          12: [(0, 6), (6, 12)],
                                          16: [(0, 8), (8, 16)],
                                          0: []}[nsub]
                                for c0, c1 in chunks:
                                    if not (c1 <= tdone and c1 > t):
                                        continue
                                    pc = tcpool.tile([128, 512], f32r,
                                                     tag="pc")
                                    dst = tsum if c0 == 0 else pc
                                    nc.vector.tensor_reduce(
                                        out=dst,
                                        in_=probs[:, :, c0:c1],
                                        axis=AX.X, op=ALU.add)
                                    if c0 != 0:
                                        nc.vector.tensor_tensor(
                                            out=tsum, in0=tsum, in1=pc,
                                            op=ALU.add)
                                # diagonal t's: for qsb0 the denominator is
                                # summed on PE (DVE is the bottleneck there)
                                for tt in range(t, tdone):
                                    if not (causal and tt >= nsub):
                                        continue
                                    ql = max(0, tt * 128 - q0g)
                                    if nsub == 0:
                                        nc.tensor.matmul(
                                            dsr0[0:1, ql:512], ones_bf,
                                            probs[:, ql:512, tt],
                                            start=(tt == 0),
                                            stop=(tt == maxkt - 1),
                                            skip_group_check=True)
                                    elif tt == nsub and nsub == 0:
                                        deng.tensor_copy(
                                            out=tsum, in_=probs[:, :, tt])
                                    else:
                                        deng.tensor_tensor(
                                            out=tsum[:, ql:512],
                                            in0=tsum[:, ql:512],
                                            in1=probs[:, ql:512, tt],
                                            op=ALU.add)
                                if pending is not None:
                                    # deferred denominator work for the
                                    # previous head
                                    finish_head(*pending)
                                    pending = None
                            if not causal or nsub > 0:
                                tsum2 = tsum
                            lp.__exit__(None, None, None)
                            denom, dkind = ((dsr0, "row")
                                            if causal and nsub == 0
                                            else (tsum2, "tile"))
                            # AV accumulate (before the denominator matmuls
                            # so PE never waits on the DVE t-sum)
                            av = pavp.tile([128, 512], f32, tag="av")
                            for t in range(maxkt):
                                ql = max(0, t * 128 - q0g) if causal else 0
                                nc.tensor.matmul(
                                    av[:, ql:512], vsb[g][:, t, :],
                                    probs[:, ql:512, t],
                                    start=(t == 0), stop=(t == maxkt - 1),
                                    skip_group_check=True)
                            pending = (att, h, av, denom, dkind)
                            # interleave wo blocks of the previous qsb
                            # (none at h0: its att isn't complete until the
                            # deferred finish of the last head lands)
                            if prev_att is not None and h > 0:
                                sched = [0, 0, 2, 4, 6, 8, 10, 13, 16]
                                for m in range(sched[h], sched[h + 1]):
                                    wo_block(m, qsb - 1, prev_att)
                    prev_att = att
                # flush the last head's denominators + trailing wo
                if pending is not None:
                    finish_head(*pending)
                    pending = None
                for m in range(KT):
                    wo_block(m, QSB - 1, prev_att)

    nc.compile()
    return nc


def _get_nc(causal: bool):
    if causal not in _compiled:
        _compiled[causal] = _build(causal)
    return _compiled[causal]


def kernel(x, freqs_cis, mask, wq, wk, wv, wo):
    from concourse.bass_utils import run_bass_kernel_spmd
    import ml_dtypes

    bf = ml_dtypes.bfloat16
    x = np.asarray(x, dtype=np.float32)
    freqs_cis = np.asarray(freqs_cis, dtype=np.float32)
    mask = np.asarray(mask, dtype=np.float32)
    wq = np.asarray(wq, dtype=np.float32)
    wk = np.asarray(wk, dtype=np.float32)
    wv = np.asarray(wv, dtype=np.float32)
    wo = np.asarray(wo, dtype=np.float32)

    tri = np.tril(np.ones((S, S), dtype=bool))
    causal = bool((mask[tri] == 0.0).all() and (mask[~tri] < -1e30).all())
    if not causal and not (mask == 0.0).all():
        return _numpy_ref(x, freqs_cis, mask, wq, wk, wv, wo)

    nc = _get_nc(causal)

    cos = freqs_cis[:, :, 0]
    sin = freqs_cis[:, :, 1]
    cosS = np.ascontiguousarray(cos.reshape(SB, 128, 64).transpose(1, 0, 2))
    sinS = np.ascontiguousarray(sin.reshape(SB, 128, 64).transpose(1, 0, 2))
    mtile = (np.ascontiguousarray(mask[0:128, 0:128].T) if causal
             else np.zeros((128, 128), dtype=np.float32))
    tri01 = np.triu(np.ones((128, 128), dtype=np.float32)).astype(bf)
    onest = np.ones((128, 128), dtype=np.float32)

    in_maps = []
    for c in range(8):
        b, i = c // 2, c % 2
        in_maps.append({
            "xT": np.ascontiguousarray(x[b].T).astype(bf),
            "wqT": np.ascontiguousarray(
                wq[1024 * i:1024 * (i + 1), :].T).astype(bf),
            "wkvT": np.ascontiguousarray(np.concatenate(
                [wk[256 * i:256 * (i + 1), :].T,
                 wv[256 * i:256 * (i + 1), :].T], axis=1)).astype(bf),
            "woT": np.ascontiguousarray(
                wo[:, 1024 * i:1024 * (i + 1)].T).astype(bf),
            "cosS": cosS, "sinS": sinS, "mtile": mtile, "onest": onest,
            "tri01": tri01,
        })

    res = run_bass_kernel_spmd(nc, in_maps, core_ids=list(range(8)))
    out = np.empty((B, S, D), dtype=np.float32)
    for b in range(B):
        out[b] = (res.results[2 * b]["outT"].astype(np.float32).T
                  + res.results[2 * b + 1]["outT"].astype(np.float32).T)
    return out


def _numpy_ref(x, freqs_cis, mask, wq, wk, wv, wo):
    xq = (x @ wq.T).reshape(B, S, H, HD)
    xk = (x @ wk.T).reshape(B, S, KV, HD)
    xv = (x @ wv.T).reshape(B, S, KV, HD)

    def rope(xh):
        x2 = xh.reshape(*xh.shape[:-1], HD // 2, 2)
        fc = freqs_cis[None, :, None, :, :]
        real = x2[..., 0] * fc[..., 0] - x2[..., 1] * fc[..., 1]
        imag = x2[..., 0] * fc[..., 1] + x2[..., 1] * fc[..., 0]
        return np.concatenate([real, imag], axis=-1)

    xq, xk = rope(xq), rope(xk)
    q = xq.reshape(B, S, KV, NREP, HD)
    sc = np.einsum('bqgrd,bkgd->bgrqk', q, xk) * SCALE + mask[None, None, None]
    sc = sc - sc.max(axis=-1, keepdims=True)
    p = np.exp(sc)
    p /= p.sum(axis=-1, keepdims=True)
    o = np.einsum('bgrqk,bkgd->bqgrd', p, xv).reshape(B, S, H * HD)
    return (o @ wo.T).astype(np.float32)
